# revision 1
# baseline (speedup 1.0000x reference)
"""Trainium2 Bass kernel for nn_Attention (B=8, N=1024, C=768, H=12).

Data-parallel over batch: core b handles batch element b.

Math (re-associated to avoid the huge bhqk,bhqd->bkd contraction):
  q = x Wq^T, k = x Wk^T             (per head h: qh, kh  [N, Z])
  S_h = qh kh^T * scale              [N, N]
  E_h = exp(S_h)   (scores are in [-3, 3]; no max-subtraction needed)
  den[qi] = sum_ki E_h[qi, ki]
  ks = kh / den[:, None], qs = qh / den[:, None]
  AT_h = [E_h^T ks ; E_h^T qs]^T     [2Z, N]   (A1T/A2T stacked)
  O    = sum_h [Wq_h ; Wk_h]^T-contracted with AT_h   (outT, [C, N])
  final = O^T Wp^T + bp              (+ rank-1 ones x bp matmul for bias)

PSUM discipline: one pool, two tags ("ps0", "ps1"), each 2 bufs of
[128, 1024] fp32 = 4 banks -> 8 banks total, shared across phases.
SBUF singles are freed in strict LIFO order between phases.

Schedule: scores/exp of pair 0 start right after qT/kT tiles 0-1; the
natural-layout projections and later qT/kT tiles are emitted inside the
ACT-bound attention pairs so TensorE fills the exp-paced stretches.
"""

import sys
from contextlib import ExitStack

import numpy as np

if "/opt/trn_rl_repo" not in sys.path:
    sys.path.insert(0, "/opt/trn_rl_repo")

import ml_dtypes
import concourse.bass as bass
import concourse.mybir as mybir
import concourse.tile as tile
from concourse import bacc, bass_utils
from concourse.bass import ts

B, N, C, H = 8, 1024, 768, 12
Z = C // H          # 64
P = 128
NT = N // P         # 8 qi tiles
CT = C // P         # 6 c tiles
SCALE = Z ** -0.5   # 0.125
FP = mybir.dt.float32
BF = mybir.dt.bfloat16
FPR = mybir.dt.float32r

CCH = [(0, 512), (512, 256)]  # C=768 split into matmul free-dim chunks

last_results = None  # set by kernel() for test harness introspection


def _r(ap):
    """bitcast to float32r for full-rate fp32 matmuls (fp32 data only)."""
    if ap.dtype == FP:
        return ap.bitcast(FPR)
    return ap


def emit(ctx: ExitStack, tc: tile.TileContext, io):
    nc = tc.nc
    xT, wqkT, WpT, Wqb, Wkb, bp, ones, out = io

    stack = []  # (name, free) in creation order; freed strictly LIFO

    def single(shape, dtype, name):
        t, free = tc.tile(shape, dtype, name=name)
        stack.append((name, free))
        return t

    def free_through(name):
        while stack:
            nm, fr = stack.pop()
            fr()
            if nm == name:
                return
        raise KeyError(name)

    # ---------------- PSUM pool: 2 tags x 2 bufs x [128,1024] = 8 banks ----
    psum = ctx.enter_context(tc.tile_pool(name="psum", bufs=2, space="PSUM"))
    _chain = [0]

    def ps_tile(tag=None):
        if tag is None:
            tag = f"ps{_chain[0] & 1}"
            _chain[0] += 1
        return psum.tile([P, N], FP, name=tag, tag=tag)

    # SBUF pools (entered before any single so LIFO holds at ctx exit)
    p_E = ctx.enter_context(tc.tile_pool(name="p_E", bufs=26))
    p_kqs = ctx.enter_context(tc.tile_pool(name="p_kqs", bufs=10))
    p_den = ctx.enter_context(tc.tile_pool(name="p_den", bufs=7))
    p_out = ctx.enter_context(tc.tile_pool(name="p_out", bufs=3))

    # ------------- singles, bottom of stack = longest-lived -------------
    W2_all = single([P, H * C], BF, name="W2_all")
    W2_sb = [W2_all[:, ts(h, C)] for h in range(H)]
    AT_sb = [single([P, N], BF, name=f"AT{h}") for h in range(H)]
    bp_sb = single([1, C], FPR, name="bp_sb")
    ones_sb = single([1, P], FPR, name="ones_sb")
    WpT_all = single([P, CT * C], FPR, name="WpT_all")
    WpT_sb = [WpT_all[:, ts(i, C)] for i in range(CT)]
    # kq_nat[t]: [128, 2C]  cols 0:C = k natural, C:2C = q natural
    kq_nat = [single([P, 2 * C], BF, name=f"kqnat{t}") for t in range(NT)]
    # qT/kT tile j: [128, N] rows = c_out 128j..128j+127 (heads 2j, 2j+1)
    qT_sb = [single([P, N], BF, name=f"qT{j}") for j in range(CT)]
    kT_sb = [single([P, N], BF, name=f"kT{j}") for j in range(CT)]
    wqkT_all = single([P, CT * 2 * C], BF, name="wqkT_all")
    wqkT_sb = [wqkT_all[:, ts(i, 2 * C)] for i in range(CT)]
    xT_all = single([P, CT * N], BF, name="xT_all")
    xT_sb = [xT_all[:, ts(i, N)] for i in range(CT)]

    # ---------------- batched input DMAs (phase-A inputs first) ---------
    for k in range(CT):
        nc.sync.dma_start(xT_sb[k][:], xT[ts(k, P), :])
        nc.sync.dma_start(wqkT_sb[k][:], wqkT[ts(k, P), :])
    nc.sync.dma_start(W2_all[0:Z, :].rearrange("z (h c) -> z h c", c=C),
                      Wqb.rearrange("(h z) c -> z h c", z=Z))
    nc.sync.dma_start(W2_all[Z:P, :].rearrange("z (h c) -> z h c", c=C),
                      Wkb.rearrange("(h z) c -> z h c", z=Z))
    nc.sync.dma_start(WpT_all[:].rearrange("p (k c) -> p k c", k=CT),
                      WpT.rearrange("(k p) c -> p k c", p=P))
    nc.sync.dma_start(bp_sb[:], bp[:])
    nc.sync.dma_start(ones_sb[:], ones[:])

    # ---------------- projection chains ----------------
    def chain(dst_ap, lhsT_of, rhs_of, width, tag=None):
        """dst_ap = sum_k lhsT_of(k)^T @ rhs_of(k); psum chain + DVE copy."""
        ps = ps_tile(tag)
        for k in range(CT):
            nc.tensor.matmul(
                ps[:, 0:width],
                lhsT=_r(lhsT_of(k)),
                rhs=_r(rhs_of(k)),
                start=(k == 0),
                stop=(k == CT - 1),
            )
        nc.vector.tensor_copy(dst_ap, ps[:, 0:width])

    def make_qkT(j, tag=None):
        # k chains + q-ch0 first: pair j's scores t=0..3 become ready one
        # chain earlier (they read kT fully but only qT cols 0:512)
        order = [(1, 0), (0, 0), (1, 1), (0, 1)]
        for which, ch in order:
            cols = slice(512 * ch, 512 * ch + 512)
            dst = (qT_sb if which == 0 else kT_sb)[j][:, cols]
            woff = C * which
            chain(dst,
                  lambda k: wqkT_sb[k][:, woff + 128 * j: woff + 128 * j + P],
                  lambda k: xT_sb[k][:, cols], 512, tag)

    def make_nat(t, tag=None):
        for half in range(2):  # 0 -> k (WkT cols), 1 -> q (WqT cols)
            woff = C * (1 - half)
            for off, w in CCH:
                chain(kq_nat[t][:, half * C + off: half * C + off + w],
                      lambda k: xT_sb[k][:, ts(t, P)],
                      lambda k, woff=woff, off=off, w=w:
                          wqkT_sb[k][:, woff + off: woff + off + w],
                      w, tag)

    # qT/kT for pair 0 up front so scores/exp start as early as possible
    make_qkT(0)

    # ---------------- phase B + interleaved remaining projections -------
    at_queue = []
    LAG = 18

    def drain_at(n):
        while len(at_queue) > n:
            at_queue.pop(0)()

    at_ps = {}

    # extra PE work emitted inside each pair (fills exp-paced stretches).
    # NB: trace order defines dependencies -- every producer must be
    # emitted before its first reader. kq_nat[t] is read by the lagged
    # scale ops (LAG/2 t-steps behind), qT/kT[j] by pair j's scores.
    extras = {0: [lambda: make_qkT(1, "ps0"), lambda: make_nat(0),
                  lambda: make_nat(1), lambda: make_nat(2)],
              1: [lambda: make_nat(3), lambda: make_nat(4),
                  lambda: make_nat(5), lambda: make_nat(6),
                  lambda: make_nat(7), lambda: make_qkT(2, "ps0")],
              2: [lambda: make_qkT(3, "ps0")],
              3: [lambda: make_qkT(4, "ps0")],
              4: [lambda: make_qkT(5, "ps0")]}

    for j in range(H // 2):
        heads = (2 * j, 2 * j + 1)
        qt, kt = qT_sb[j], kT_sb[j]
        den_t = {h: p_den.tile([P, NT], FP, name="dent") for h in heads}
        rv_t = {h: p_den.tile([P, NT], FP, name="rvt") for h in heads}
        for h in heads:
            at_ps[h] = ps_tile("ps1")
        ext = extras.get(j, [])
        for t in range(NT):
            S = {h: ps_tile("ps0") for h in heads}
            for ch in range(2):
                cols = slice(512 * ch, 512 * ch + 512)
                for h in heads:
                    base = Z * (h & 1)
                    nc.tensor.matmul(
                        S[h][:, cols],
                        lhsT=qt[base:base + Z, ts(t, P)],
                        rhs=kt[base:base + Z, cols],
                        start=True, stop=True,
                    )
            for h in heads:
                E = p_E.tile([P, N], BF, name="Et")
                nc.scalar.activation(
                    E[:], S[h][:], mybir.ActivationFunctionType.Exp,
                    scale=SCALE, accum_out=den_t[h][:, t:t + 1],
                )

                def at_mm(h=h, t=t, E=E, den_t=den_t, rv_t=rv_t):
                    nc.vector.reciprocal(rv_t[h][:, t:t + 1],
                                         den_t[h][:, t:t + 1])
                    kqs = p_kqs.tile([P, 2 * Z], BF, name="kqst")
                    nc.vector.tensor_scalar_mul(
                        kqs[:].rearrange("p (g z) -> p g z", g=2),
                        kq_nat[t].rearrange("p (g c) -> p g c", g=2)
                        [:, :, ts(h, Z)],
                        rv_t[h][:, t:t + 1],
                    )
                    for ch in range(2):
                        cols = slice(512 * ch, 512 * ch + 512)
                        nc.tensor.matmul(
                            at_ps[h][:, cols],
                            lhsT=kqs[:],
                            rhs=E[:, cols],
                            start=(t == 0), stop=(t == NT - 1),
                        )

                at_queue.append(at_mm)
                drain_at(LAG)
            if ext:
                ext.pop(0)()
        for h in heads:
            def at_copy(h=h):
                nc.vector.tensor_copy(AT_sb[h][:], at_ps.pop(h)[:])
            at_queue.append(at_copy)
    drain_at(0)

    free_through("kqnat0")  # frees xT, wqkT, kT*, qT*, kqnat*

    # ---------------- phase C: combine over heads, project, bias ------
    O_sb = [single([P, N], FPR, name=f"O{d}") for d in range(CT)]
    for d in range(CT):
        O_ps = ps_tile("ps0")
        for ch in range(2):
            cols = slice(512 * ch, 512 * ch + 512)
            for h in range(H):
                nc.tensor.matmul(
                    O_ps[:, cols],
                    lhsT=W2_sb[h][:, ts(d, P)],
                    rhs=AT_sb[h][:, cols],
                    start=(h == 0), stop=(h == H - 1),
                )
        nc.vector.tensor_copy(O_sb[d][:], O_ps[:])

    for t in range(NT):
        F_ps = ps_tile("ps1")
        for off, w in CCH:
            for k in range(CT):
                nc.tensor.matmul(
                    F_ps[:, off:off + w],
                    lhsT=_r(O_sb[k][:, ts(t, P)]),
                    rhs=_r(WpT_sb[k][:, off:off + w]),
                    start=(k == 0), stop=False,
                )
            # + ones^T x bp  (rank-1 bias add inside the accumulation)
            nc.tensor.matmul(
                F_ps[:, off:off + w],
                lhsT=_r(ones_sb[:, 0:P]),
                rhs=_r(bp_sb[:, off:off + w]),
                start=False, stop=True,
            )
        o = p_out.tile([P, C], FP, name="outt")
        if t % 2 == 0:
            nc.scalar.copy(o[:], F_ps[:, 0:C])
        else:
            nc.vector.tensor_copy(o[:], F_ps[:, 0:C])
        nc.sync.dma_start(out[ts(t, P), :], o[:])

    while stack:
        stack.pop()[1]()


def build():
    nc = bacc.Bacc("TRN2", target_bir_lowering=False, debug=False, num_devices=B)
    xT = nc.dram_tensor("xT", [C, N], BF, kind="ExternalInput").ap()
    wqkT = nc.dram_tensor("wqkT", [C, 2 * C], BF, kind="ExternalInput").ap()
    Wqb = nc.dram_tensor("Wqb", [C, C], BF, kind="ExternalInput").ap()
    Wkb = nc.dram_tensor("Wkb", [C, C], BF, kind="ExternalInput").ap()
    WpT = nc.dram_tensor("WpT", [C, C], FPR, kind="ExternalInput").ap()
    bp = nc.dram_tensor("bp", [1, C], FPR, kind="ExternalInput").ap()
    ones = nc.dram_tensor("ones", [1, P], FPR, kind="ExternalInput").ap()
    out = nc.dram_tensor("out", [N, C], FP, kind="ExternalOutput").ap()
    with tile.TileContext(nc) as tc, ExitStack() as ctx:
        emit(ctx, tc, (xT, wqkT, WpT, Wqb, Wkb, bp, ones, out))
    nc.compile()
    return nc


def kernel(x, Wq, Wk, Wp, bp, trace=False, **trace_kwargs):
    global last_results
    x = np.asarray(x, dtype=np.float32)
    Wq = np.asarray(Wq, dtype=np.float32)
    Wk = np.asarray(Wk, dtype=np.float32)
    Wp = np.asarray(Wp, dtype=np.float32)
    bp = np.asarray(bp, dtype=np.float32)

    nc = build()
    bf = ml_dtypes.bfloat16
    wqkTc = np.ascontiguousarray(
        np.concatenate([Wq.T, Wk.T], axis=1)).astype(bf)  # [C, 2C]
    Wqbc = np.ascontiguousarray(Wq).astype(bf)
    Wkbc = np.ascontiguousarray(Wk).astype(bf)
    WpTc = np.ascontiguousarray(Wp.T)                     # [C, C] fp32
    bpc = np.ascontiguousarray(bp.reshape(1, C))
    onesc = np.ones((1, P), dtype=np.float32)
    in_maps = []
    for b in range(B):
        in_maps.append({
            "xT": np.ascontiguousarray(x[b].T).astype(bf),
            "wqkT": wqkTc, "Wqb": Wqbc, "Wkb": Wkbc,
            "WpT": WpTc, "bp": bpc, "ones": onesc,
        })
    res = bass_utils.run_bass_kernel_spmd(
        nc, in_maps, core_ids=list(range(B)), trace=trace, **trace_kwargs)
    last_results = res
    return np.stack([res.results[b]["out"] for b in range(B)], axis=0)



# revision 2
# speedup vs baseline: 1.0916x; 1.0916x over previous
"""Trainium2 Bass kernel for nn_Attention (B=8, N=1024, C=768, H=12).

Data-parallel over batch: core b handles batch element b.

Math (re-associated to avoid the huge bhqk,bhqd->bkd contraction):
  q = x Wq^T, k = x Wk^T             (per head h: qh, kh  [N, Z])
  S_h = qh kh^T * scale              [N, N]
  E_h = exp(S_h)   (scores are in [-3, 3]; no max-subtraction needed)
  den[qi] = sum_ki E_h[qi, ki]
  ks = kh / den[:, None], qs = qh / den[:, None]
  AT_h = [E_h^T ks ; E_h^T qs]^T     [2Z, N]   (A1T/A2T stacked)
  out  = sum_h AT_h^T @ M_hT + bp    with M_h = [Wq_h;Wk_h] @ Wp^T
         (head-combine and output projection fused on the host)

v2 changes vs baseline:
  - natural-layout q/k (for the 1/den scaling) no longer recomputed by
    matmul; qT/kT are round-tripped through DRAM and transposed by the
    DMA xbar into natkq[j] while the PE does real work.
  - phase C collapsed: F[t] = sum_h AT_h[:,t]^T @ M_hT (96+96 MMs)
    replaces combine(144) + final fp32 (96) + bias (16) matmuls; bias
    is added by DVE during the PSUM->SBUF copy against a replicated
    [128, C] bias tile.

PSUM discipline: one pool, two tags ("ps0", "ps1"), each 2 bufs of
[128, 1024] fp32 = 4 banks -> 8 banks total, shared across phases.
SBUF singles are freed in strict LIFO order between phases.
"""

import sys
from contextlib import ExitStack

import numpy as np

if "/opt/trn_rl_repo" not in sys.path:
    sys.path.insert(0, "/opt/trn_rl_repo")

import ml_dtypes
import concourse.bass as bass
import concourse.mybir as mybir
import concourse.tile as tile
from concourse import bacc, bass_utils
from concourse.bass import ts

B, N, C, H = 8, 1024, 768, 12
Z = C // H          # 64
P = 128
NT = N // P         # 8 qi tiles
CT = C // P         # 6 c tiles
SCALE = Z ** -0.5   # 0.125
FP = mybir.dt.float32
BF = mybir.dt.bfloat16
FPR = mybir.dt.float32r

CCH = [(0, 512), (512, 256)]  # C=768 split into matmul free-dim chunks

last_results = None  # set by kernel() for test harness introspection


def _r(ap):
    """bitcast to float32r for full-rate fp32 matmuls (fp32 data only)."""
    if ap.dtype == FP:
        return ap.bitcast(FPR)
    return ap


def emit(ctx: ExitStack, tc: tile.TileContext, io):
    nc = tc.nc
    xT, wqkT, M, bpr, out = io

    stack = []  # (name, free) in creation order; freed strictly LIFO

    def single(shape, dtype, name):
        t, free = tc.tile(shape, dtype, name=name)
        stack.append((name, free))
        return t

    def free_through(name):
        while stack:
            nm, fr = stack.pop()
            fr()
            if nm == name:
                return
        raise KeyError(name)

    # ---------------- PSUM pool: 2 tags x 2 bufs x [128,1024] = 8 banks ----
    psum = ctx.enter_context(tc.tile_pool(name="psum", bufs=2, space="PSUM"))
    _chain = [0]

    def ps_tile(tag=None):
        if tag is None:
            tag = f"ps{_chain[0] & 1}"
            _chain[0] += 1
        return psum.tile([P, N], FP, name=tag, tag=tag)

    # SBUF pools (entered before any single so LIFO holds at ctx exit)
    p_E = ctx.enter_context(tc.tile_pool(name="p_E", bufs=26))
    p_kqs = ctx.enter_context(tc.tile_pool(name="p_kqs", bufs=10))
    p_den = ctx.enter_context(tc.tile_pool(name="p_den", bufs=7))
    p_out = ctx.enter_context(tc.tile_pool(name="p_out", bufs=3))

    # ------------- singles, bottom of stack = longest-lived -------------
    M_all = single([P, H * C], BF, name="M_all")
    M_sb = [M_all[:, ts(h, C)] for h in range(H)]
    bp_sb = single([P, C], FP, name="bp_sb")
    AT_sb = [single([P, N], BF, name=f"AT{h}") for h in range(H)]
    # natkq[j]: [128, 2N] cols 0:N = k natural (t-major 128-col blocks),
    # N:2N = q natural; features c of heads 2j, 2j+1.
    natkq = [single([P, 2 * N], BF, name=f"natkq{j}") for j in range(CT)]
    # qT/kT tile j: [128, N] rows = c_out 128j..128j+127 (heads 2j, 2j+1)
    qT_sb = [single([P, N], BF, name=f"qT{j}") for j in range(CT)]
    kT_sb = [single([P, N], BF, name=f"kT{j}") for j in range(CT)]
    wqkT_all = single([P, CT * 2 * C], BF, name="wqkT_all")
    wqkT_sb = [wqkT_all[:, ts(i, 2 * C)] for i in range(CT)]
    xT_all = single([P, CT * N], BF, name="xT_all")
    xT_sb = [xT_all[:, ts(i, N)] for i in range(CT)]

    # DRAM scratch for the qT/kT -> natural-layout xbar transposes
    qkTd = []
    for j in range(CT):
        t_, _free = tc.tile([2, P, N], BF, space="DRAM", name=f"qkTd{j}")
        qkTd.append(t_)

    # ---------------- batched input DMAs (phase-A inputs first) ---------
    for k in range(CT):
        nc.sync.dma_start(xT_sb[k][:], xT[ts(k, P), :])
        nc.sync.dma_start(wqkT_sb[k][:], wqkT[ts(k, P), :])
    # phase-C inputs ride the Activation hwdge queue (idle until exp starts)
    nc.scalar.dma_start(M_all[:], M[:])
    nc.scalar.dma_start(bp_sb[:], bpr[:])

    # ---------------- projection chains ----------------
    def chain(dst_ap, lhsT_of, rhs_of, width, tag=None):
        """dst_ap = sum_k lhsT_of(k)^T @ rhs_of(k); psum chain + DVE copy."""
        ps = ps_tile(tag)
        for k in range(CT):
            nc.tensor.matmul(
                ps[:, 0:width],
                lhsT=_r(lhsT_of(k)),
                rhs=_r(rhs_of(k)),
                start=(k == 0),
                stop=(k == CT - 1),
            )
        nc.vector.tensor_copy(dst_ap, ps[:, 0:width])

    def make_qkT(j, tag=None):
        # k chains + q-ch0 first: pair j's scores t=0..3 become ready one
        # chain earlier (they read kT fully but only qT cols 0:512)
        order = [(1, 0), (0, 0), (1, 1), (0, 1)]
        for which, ch in order:
            cols = slice(512 * ch, 512 * ch + 512)
            dst = (qT_sb if which == 0 else kT_sb)[j][:, cols]
            woff = C * which
            chain(dst,
                  lambda k: wqkT_sb[k][:, woff + 128 * j: woff + 128 * j + P],
                  lambda k: xT_sb[k][:, cols], 512, tag)

    def emit_nat_dma(j):
        """qT/kT[j] -> DRAM -> xbar-transposed natural layout natkq[j]."""
        nc.sync.dma_start(qkTd[j][1], kT_sb[j][:])
        nc.sync.dma_start(qkTd[j][0], qT_sb[j][:])
        nc.sync.dma_start_transpose(
            natkq[j][:, 0:N].rearrange("p (t c) -> p t c", c=P),
            qkTd[j][1].rearrange("c (t q) -> c t q", q=P))
        nc.sync.dma_start_transpose(
            natkq[j][:, N:2 * N].rearrange("p (t c) -> p t c", c=P),
            qkTd[j][0].rearrange("c (t q) -> c t q", q=P))

    # qT/kT for pair 0 up front so scores/exp start as early as possible
    make_qkT(0)
    emit_nat_dma(0)

    # ---------------- phase B + interleaved remaining projections -------
    at_queue = []
    LAG = 18

    def drain_at(n):
        while len(at_queue) > n:
            at_queue.pop(0)()

    at_ps = {}

    # extra PE work emitted inside each pair (fills exp-paced stretches).
    # NB: trace order defines dependencies -- every producer must be
    # emitted before its first reader. natkq[j] is read by the lagged
    # scale ops of pair j; qT/kT[j] by pair j's scores.
    extras = {j: [lambda j=j: make_qkT(j + 1, "ps0"),
                  lambda j=j: emit_nat_dma(j + 1)]
              for j in range(5)}

    for j in range(H // 2):
        heads = (2 * j, 2 * j + 1)
        qt, kt = qT_sb[j], kT_sb[j]
        nat3 = natkq[j].rearrange("p (g t c) -> p g t c", g=2, c=P)
        den_t = {h: p_den.tile([P, NT], FP, name="dent") for h in heads}
        rv_t = {h: p_den.tile([P, NT], FP, name="rvt") for h in heads}
        for h in heads:
            at_ps[h] = ps_tile("ps1")
        ext = extras.get(j, [])
        for t in range(NT):
            S = {h: ps_tile("ps0") for h in heads}
            for ch in range(2):
                cols = slice(512 * ch, 512 * ch + 512)
                for h in heads:
                    base = Z * (h & 1)
                    nc.tensor.matmul(
                        S[h][:, cols],
                        lhsT=qt[base:base + Z, ts(t, P)],
                        rhs=kt[base:base + Z, cols],
                        start=True, stop=True,
                    )
            for h in heads:
                E = p_E.tile([P, N], BF, name="Et")
                nc.scalar.activation(
                    E[:], S[h][:], mybir.ActivationFunctionType.Exp,
                    scale=SCALE, accum_out=den_t[h][:, t:t + 1],
                )

                def at_mm(h=h, t=t, E=E, den_t=den_t, rv_t=rv_t, nat3=nat3):
                    nc.vector.reciprocal(rv_t[h][:, t:t + 1],
                                         den_t[h][:, t:t + 1])
                    kqs = p_kqs.tile([P, 2 * Z], BF, name="kqst")
                    nc.vector.tensor_scalar_mul(
                        kqs[:].rearrange("p (g z) -> p g z", g=2),
                        nat3[:, :, t, ts(h & 1, Z)],
                        rv_t[h][:, t:t + 1],
                    )
                    for ch in range(2):
                        cols = slice(512 * ch, 512 * ch + 512)
                        nc.tensor.matmul(
                            at_ps[h][:, cols],
                            lhsT=kqs[:],
                            rhs=E[:, cols],
                            start=(t == 0), stop=(t == NT - 1),
                        )

                at_queue.append(at_mm)
                drain_at(LAG)
            if ext:
                ext.pop(0)()
        for h in heads:
            def at_copy(h=h):
                nc.vector.tensor_copy(AT_sb[h][:], at_ps.pop(h)[:])
            at_queue.append(at_copy)
    drain_at(0)

    free_through("natkq0")  # frees xT, wqkT, kT*, qT*, natkq*

    # ---------------- phase C: fused combine + projection + bias ------
    for t in range(NT):
        F_ps = ps_tile()
        for h in range(H):
            for off, w in CCH:
                nc.tensor.matmul(
                    F_ps[:, off:off + w],
                    lhsT=AT_sb[h][:, ts(t, P)],
                    rhs=M_sb[h][:, off:off + w],
                    start=(h == 0), stop=(h == H - 1),
                )
        o = p_out.tile([P, C], FP, name="outt")
        nc.vector.tensor_add(o[:], F_ps[:, 0:C], bp_sb[:])
        nc.sync.dma_start(out[ts(t, P), :], o[:])

    while stack:
        stack.pop()[1]()


def build():
    nc = bacc.Bacc("TRN2", target_bir_lowering=False, debug=False, num_devices=B)
    xT = nc.dram_tensor("xT", [C, N], BF, kind="ExternalInput").ap()
    wqkT = nc.dram_tensor("wqkT", [C, 2 * C], BF, kind="ExternalInput").ap()
    M = nc.dram_tensor("M", [P, H * C], BF, kind="ExternalInput").ap()
    bpr = nc.dram_tensor("bpr", [P, C], FP, kind="ExternalInput").ap()
    out = nc.dram_tensor("out", [N, C], FP, kind="ExternalOutput").ap()
    with tile.TileContext(nc) as tc, ExitStack() as ctx:
        emit(ctx, tc, (xT, wqkT, M, bpr, out))
    nc.compile()
    return nc


def kernel(x, Wq, Wk, Wp, bp, trace=False, **trace_kwargs):
    global last_results
    x = np.asarray(x, dtype=np.float32)
    Wq = np.asarray(Wq, dtype=np.float32)
    Wk = np.asarray(Wk, dtype=np.float32)
    Wp = np.asarray(Wp, dtype=np.float32)
    bp = np.asarray(bp, dtype=np.float32)

    nc = build()
    bf = ml_dtypes.bfloat16
    wqkTc = np.ascontiguousarray(
        np.concatenate([Wq.T, Wk.T], axis=1)).astype(bf)  # [C, 2C]
    # fused combine+projection weights: M_hT = [Wq_h; Wk_h] @ Wp^T  [2Z, C]
    Wq_h = Wq.reshape(H, Z, C)
    Wk_h = Wk.reshape(H, Z, C)
    W2 = np.concatenate([Wq_h, Wk_h], axis=1)             # [H, 2Z, C]
    M_np = np.einsum("hzc,dc->hzd", W2, Wp)               # [H, 2Z, C]
    Mc = np.ascontiguousarray(
        M_np.transpose(1, 0, 2).reshape(P, H * C)).astype(bf)
    bprc = np.ascontiguousarray(
        np.broadcast_to(bp.reshape(1, C), (P, C)).astype(np.float32))
    in_maps = []
    for b in range(B):
        in_maps.append({
            "xT": np.ascontiguousarray(x[b].T).astype(bf),
            "wqkT": wqkTc, "M": Mc, "bpr": bprc,
        })
    res = bass_utils.run_bass_kernel_spmd(
        nc, in_maps, core_ids=list(range(B)), trace=trace, **trace_kwargs)
    last_results = res
    return np.stack([res.results[b]["out"] for b in range(B)], axis=0)


# revision 5
# speedup vs baseline: 1.2173x; 1.1152x over previous
"""Trainium2 Bass kernel for nn_Attention (B=8, N=1024, C=768, H=12).

Data-parallel over batch: core b handles batch element b.

Math (re-associated to avoid the huge bhqk,bhqd->bkd contraction):
  q = x Wq^T, k = x Wk^T             (per head h: qh, kh  [N, Z])
  S_h = qh kh^T * scale              [N, N]
  E_h = exp(S_h)   (scores are in [-3, 3]; no max-subtraction needed)
  den[qi] = sum_ki E_h[qi, ki]
  ks = kh / den[:, None], qs = qh / den[:, None]
  AT_h = [E_h^T ks ; E_h^T qs]^T     [2Z, N]   (A1T/A2T stacked)
  out  = sum_h AT_h^T @ M_hT + bp    with M_h = [Wq_h;Wk_h] @ Wp^T
         (head-combine and output projection fused on the host)

v2 changes vs baseline:
  - natural-layout q/k (for the 1/den scaling) no longer recomputed by
    matmul; qT/kT are round-tripped through DRAM and transposed by the
    DMA xbar into natkq[j] while the PE does real work.
  - phase C collapsed: F[t] = sum_h AT_h[:,t]^T @ M_hT (96+96 MMs)
    replaces combine(144) + final fp32 (96) + bias (16) matmuls; bias
    is added by DVE during the PSUM->SBUF copy against a replicated
    [128, C] bias tile.

PSUM discipline: one pool, two tags ("ps0", "ps1"), each 2 bufs of
[128, 1024] fp32 = 4 banks -> 8 banks total, shared across phases.
SBUF singles are freed in strict LIFO order between phases.
"""

import sys
from contextlib import ExitStack

import numpy as np

if "/opt/trn_rl_repo" not in sys.path:
    sys.path.insert(0, "/opt/trn_rl_repo")

import ml_dtypes
import concourse.bass as bass
import concourse.mybir as mybir
import concourse.tile as tile
from concourse import bacc, bass_utils
from concourse.bass import ts

B, N, C, H = 8, 1024, 768, 12
Z = C // H          # 64
P = 128
NT = N // P         # 8 qi tiles
CT = C // P         # 6 c tiles
SCALE = Z ** -0.5   # 0.125
FP = mybir.dt.float32
BF = mybir.dt.bfloat16
FPR = mybir.dt.float32r

CCH = [(0, 512), (512, 256)]  # C=768 split into matmul free-dim chunks

last_results = None  # set by kernel() for test harness introspection


def _r(ap):
    """bitcast to float32r for full-rate fp32 matmuls (fp32 data only)."""
    if ap.dtype == FP:
        return ap.bitcast(FPR)
    return ap


def emit(ctx: ExitStack, tc: tile.TileContext, io):
    nc = tc.nc
    xT, wqkT, M, bpr, out = io

    stack = []  # (name, free) in creation order; freed strictly LIFO

    def single(shape, dtype, name):
        t, free = tc.tile(shape, dtype, name=name)
        stack.append((name, free))
        return t

    def free_through(name):
        while stack:
            nm, fr = stack.pop()
            fr()
            if nm == name:
                return
        raise KeyError(name)

    # ---------------- PSUM pool: 2 tags x 2 bufs x [128,1024] = 8 banks ----
    psum = ctx.enter_context(tc.tile_pool(name="psum", bufs=2, space="PSUM"))
    _chain = [0]

    def ps_tile(tag=None):
        if tag is None:
            tag = f"ps{_chain[0] & 1}"
            _chain[0] += 1
        return psum.tile([P, N], FP, name=tag, tag=tag)

    # SBUF pools (entered before any single so LIFO holds at ctx exit)
    p_E = ctx.enter_context(tc.tile_pool(name="p_E", bufs=26))
    p_kqs = ctx.enter_context(tc.tile_pool(name="p_kqs", bufs=10))
    p_den = ctx.enter_context(tc.tile_pool(name="p_den", bufs=7))
    p_out = ctx.enter_context(tc.tile_pool(name="p_out", bufs=3))

    # ------------- singles, bottom of stack = longest-lived -------------
    M_all = single([P, H * C], BF, name="M_all")
    M_sb = [M_all[:, ts(h, C)] for h in range(H)]
    bp_sb = single([P, C], FP, name="bp_sb")
    AT_sb = [single([P, N], BF, name=f"AT{h}") for h in range(H)]
    # natkq[j]: [128, 2N] cols 0:N = k natural (t-major 128-col blocks),
    # N:2N = q natural; features c of heads 2j, 2j+1.
    natkq = [single([P, 2 * N], BF, name=f"natkq{j}") for j in range(CT)]
    # qT/kT tile j: [128, N] rows = c_out 128j..128j+127 (heads 2j, 2j+1)
    qT_sb = [single([P, N], BF, name=f"qT{j}") for j in range(CT)]
    kT_sb = [single([P, N], BF, name=f"kT{j}") for j in range(CT)]
    wqkT_all = single([P, CT * 2 * C], BF, name="wqkT_all")
    wqkT_sb = [wqkT_all[:, ts(i, 2 * C)] for i in range(CT)]
    xT_all = single([P, CT * N], BF, name="xT_all")
    xT_sb = [xT_all[:, ts(i, N)] for i in range(CT)]

    # DRAM scratch for the qT/kT -> natural-layout xbar transposes
    qkTd = []
    for j in range(CT):
        t_, _free = tc.tile([2, P, N], BF, space="DRAM", name=f"qkTd{j}")
        qkTd.append(t_)

    # ---------------- batched input DMAs (phase-A inputs first) ---------
    for k in range(CT):
        nc.sync.dma_start(xT_sb[k][:], xT[ts(k, P), :])
        nc.sync.dma_start(wqkT_sb[k][:], wqkT[ts(k, P), :])
    # phase-C inputs follow on the same queue (needed only much later);
    # a second hwdge queue tangles the DMA semaphore ring and stalls the
    # input stream, so everything stays on sync.
    nc.sync.dma_start(M_all[:], M[:])
    nc.sync.dma_start(bp_sb[:], bpr[:])

    # ---------------- projection chains ----------------
    def chain(dst_ap, lhsT_of, rhs_of, width, tag=None):
        """dst_ap = sum_k lhsT_of(k)^T @ rhs_of(k); psum chain + DVE copy."""
        ps = ps_tile(tag)
        for k in range(CT):
            nc.tensor.matmul(
                ps[:, 0:width],
                lhsT=_r(lhsT_of(k)),
                rhs=_r(rhs_of(k)),
                start=(k == 0),
                stop=(k == CT - 1),
            )
        nc.vector.tensor_copy(dst_ap, ps[:, 0:width])

    def qkT_chains(j, tag=None):
        # k chains + q-ch0 first: pair j's scores t=0..3 become ready one
        # chain earlier (they read kT fully but only qT cols 0:512).
        # Returns one thunk per chain so callers can spread them across
        # t-steps: bunching all 4 into one slot monopolizes the 2-buf ps0
        # ring and stalls the next t-step's score matmuls (and the ACT
        # engine behind them) for the whole bunch.
        def one(which, ch):
            cols = slice(512 * ch, 512 * ch + 512)
            dst = (qT_sb if which == 0 else kT_sb)[j][:, cols]
            woff = C * which
            chain(dst,
                  lambda k: wqkT_sb[k][:, woff + 128 * j: woff + 128 * j + P],
                  lambda k: xT_sb[k][:, cols], 512, tag)
        return [lambda w=w, c=c: one(w, c) for w, c in
                [(1, 0), (0, 0), (1, 1), (0, 1)]]

    def make_qkT(j, tag=None):
        for th in qkT_chains(j, tag):
            th()

    def emit_nat_dma(j):
        """qT/kT[j] -> DRAM -> xbar-transposed natural layout natkq[j]."""
        nc.sync.dma_start(qkTd[j][1], kT_sb[j][:])
        nc.sync.dma_start(qkTd[j][0], qT_sb[j][:])
        nc.sync.dma_start_transpose(
            natkq[j][:, 0:N].rearrange("p (t c) -> p t c", c=P),
            qkTd[j][1].rearrange("c (t q) -> c t q", q=P))
        nc.sync.dma_start_transpose(
            natkq[j][:, N:2 * N].rearrange("p (t c) -> p t c", c=P),
            qkTd[j][0].rearrange("c (t q) -> c t q", q=P))

    # qT/kT for pair 0 up front so scores/exp start as early as possible
    make_qkT(0)
    emit_nat_dma(0)

    # ---------------- phase B + interleaved remaining projections -------
    at_queue = []
    LAG = 18

    def drain_at(n):
        while len(at_queue) > n:
            at_queue.pop(0)()

    at_ps = {}

    # extra PE work emitted inside each pair (fills exp-paced stretches).
    # NB: trace order defines dependencies -- every producer must be
    # emitted before its first reader. natkq[j] is read by the lagged
    # scale ops of pair j; qT/kT[j] by pair j's scores.
    extras = {j: qkT_chains(j + 1, "ps0") + [lambda j=j: emit_nat_dma(j + 1)]
              for j in range(5)}

    for j in range(H // 2):
        heads = (2 * j, 2 * j + 1)
        qt, kt = qT_sb[j], kT_sb[j]
        nat3 = natkq[j].rearrange("p (g t c) -> p g t c", g=2, c=P)
        den_t = {h: p_den.tile([P, NT], FP, name="dent") for h in heads}
        rv_t = {h: p_den.tile([P, NT], FP, name="rvt") for h in heads}
        for h in heads:
            at_ps[h] = ps_tile("ps1")
        ext = extras.get(j, [])
        for t in range(NT):
            S = {h: ps_tile("ps0") for h in heads}
            for ch in range(2):
                cols = slice(512 * ch, 512 * ch + 512)
                for h in heads:
                    base = Z * (h & 1)
                    nc.tensor.matmul(
                        S[h][:, cols],
                        lhsT=qt[base:base + Z, ts(t, P)],
                        rhs=kt[base:base + Z, cols],
                        start=True, stop=True,
                    )
            for h in heads:
                E = p_E.tile([P, N], BF, name="Et")
                nc.scalar.activation(
                    E[:], S[h][:], mybir.ActivationFunctionType.Exp,
                    scale=SCALE, accum_out=den_t[h][:, t:t + 1],
                )

                def at_mm(h=h, t=t, E=E, den_t=den_t, rv_t=rv_t, nat3=nat3):
                    nc.vector.reciprocal(rv_t[h][:, t:t + 1],
                                         den_t[h][:, t:t + 1])
                    kqs = p_kqs.tile([P, 2 * Z], BF, name="kqst")
                    nc.vector.tensor_scalar_mul(
                        kqs[:].rearrange("p (g z) -> p g z", g=2),
                        nat3[:, :, t, ts(h & 1, Z)],
                        rv_t[h][:, t:t + 1],
                    )
                    for ch in range(2):
                        cols = slice(512 * ch, 512 * ch + 512)
                        nc.tensor.matmul(
                            at_ps[h][:, cols],
                            lhsT=kqs[:],
                            rhs=E[:, cols],
                            start=(t == 0), stop=(t == NT - 1),
                        )

                at_queue.append(at_mm)
                drain_at(LAG)
            if ext:
                ext.pop(0)()
        for h in heads:
            def at_copy(h=h):
                nc.vector.tensor_copy(AT_sb[h][:], at_ps.pop(h)[:])
            at_queue.append(at_copy)
    drain_at(0)

    free_through("natkq0")  # frees xT, wqkT, kT*, qT*, natkq*

    # ---------------- phase C: fused combine + projection + bias ------
    for t in range(NT):
        F_ps = ps_tile()
        for h in range(H):
            for off, w in CCH:
                nc.tensor.matmul(
                    F_ps[:, off:off + w],
                    lhsT=AT_sb[h][:, ts(t, P)],
                    rhs=M_sb[h][:, off:off + w],
                    start=(h == 0), stop=(h == H - 1),
                )
        o = p_out.tile([P, C], FP, name="outt")
        nc.vector.tensor_add(o[:], F_ps[:, 0:C], bp_sb[:])
        nc.sync.dma_start(out[ts(t, P), :], o[:])

    while stack:
        stack.pop()[1]()


def build():
    nc = bacc.Bacc("TRN2", target_bir_lowering=False, debug=False, num_devices=B)
    xT = nc.dram_tensor("xT", [C, N], BF, kind="ExternalInput").ap()
    wqkT = nc.dram_tensor("wqkT", [C, 2 * C], BF, kind="ExternalInput").ap()
    M = nc.dram_tensor("M", [P, H * C], BF, kind="ExternalInput").ap()
    bpr = nc.dram_tensor("bpr", [P, C], FP, kind="ExternalInput").ap()
    out = nc.dram_tensor("out", [N, C], FP, kind="ExternalOutput").ap()
    with tile.TileContext(nc) as tc, ExitStack() as ctx:
        emit(ctx, tc, (xT, wqkT, M, bpr, out))
    nc.compile()
    return nc


def kernel(x, Wq, Wk, Wp, bp, trace=False, **trace_kwargs):
    global last_results
    x = np.asarray(x, dtype=np.float32)
    Wq = np.asarray(Wq, dtype=np.float32)
    Wk = np.asarray(Wk, dtype=np.float32)
    Wp = np.asarray(Wp, dtype=np.float32)
    bp = np.asarray(bp, dtype=np.float32)

    nc = build()
    bf = ml_dtypes.bfloat16
    wqkTc = np.ascontiguousarray(
        np.concatenate([Wq.T, Wk.T], axis=1)).astype(bf)  # [C, 2C]
    # fused combine+projection weights: M_hT = [Wq_h; Wk_h] @ Wp^T  [2Z, C]
    Wq_h = Wq.reshape(H, Z, C)
    Wk_h = Wk.reshape(H, Z, C)
    W2 = np.concatenate([Wq_h, Wk_h], axis=1)             # [H, 2Z, C]
    M_np = np.einsum("hzc,dc->hzd", W2, Wp)               # [H, 2Z, C]
    Mc = np.ascontiguousarray(
        M_np.transpose(1, 0, 2).reshape(P, H * C)).astype(bf)
    bprc = np.ascontiguousarray(
        np.broadcast_to(bp.reshape(1, C), (P, C)).astype(np.float32))
    in_maps = []
    for b in range(B):
        in_maps.append({
            "xT": np.ascontiguousarray(x[b].T).astype(bf),
            "wqkT": wqkTc, "M": Mc, "bpr": bprc,
        })
    res = bass_utils.run_bass_kernel_spmd(
        nc, in_maps, core_ids=list(range(B)), trace=trace, **trace_kwargs)
    last_results = res
    return np.stack([res.results[b]["out"] for b in range(B)], axis=0)


# revision 8
# speedup vs baseline: 1.2499x; 1.0268x over previous
"""Trainium2 Bass kernel for nn_Attention (B=8, N=1024, C=768, H=12).

Data-parallel over batch: core b handles batch element b.

Math (re-associated to avoid the huge bhqk,bhqd->bkd contraction):
  q = x Wq^T, k = x Wk^T             (per head h: qh, kh  [N, Z])
  S_h = qh kh^T * scale              [N, N]
  E_h = exp(S_h)   (scores are in [-3, 3]; no max-subtraction needed)
  den[qi] = sum_ki E_h[qi, ki]
  ks = kh / den[:, None], qs = qh / den[:, None]
  AT_h = [E_h^T ks ; E_h^T qs]^T     [2Z, N]   (A1T/A2T stacked)
  out  = sum_h AT_h^T @ M_hT + bp    with M_h = [Wq_h;Wk_h] @ Wp^T
         (head-combine and output projection fused on the host)

v2 changes vs baseline:
  - natural-layout q/k (for the 1/den scaling) no longer recomputed by
    matmul; qT/kT are round-tripped through DRAM and transposed by the
    DMA xbar into natkq[j] while the PE does real work.
  - phase C collapsed: F[t] = sum_h AT_h[:,t]^T @ M_hT (96+96 MMs)
    replaces combine(144) + final fp32 (96) + bias (16) matmuls; bias
    is added by DVE during the PSUM->SBUF copy against a replicated
    [128, C] bias tile.

PSUM discipline: one pool, two tags ("ps0", "ps1"), each 2 bufs of
[128, 1024] fp32 = 4 banks -> 8 banks total, shared across phases.
SBUF singles are freed in strict LIFO order between phases.
"""

import sys
from contextlib import ExitStack

import numpy as np

if "/opt/trn_rl_repo" not in sys.path:
    sys.path.insert(0, "/opt/trn_rl_repo")

import ml_dtypes
import concourse.bass as bass
import concourse.mybir as mybir
import concourse.tile as tile
from concourse import bacc, bass_utils
from concourse.bass import ts

B, N, C, H = 8, 1024, 768, 12
Z = C // H          # 64
P = 128
NT = N // P         # 8 qi tiles
CT = C // P         # 6 c tiles
SCALE = Z ** -0.5   # 0.125
FP = mybir.dt.float32
BF = mybir.dt.bfloat16
FPR = mybir.dt.float32r

CCH = [(0, 512), (512, 256)]  # C=768 split into matmul free-dim chunks

last_results = None  # set by kernel() for test harness introspection


def _r(ap):
    """bitcast to float32r for full-rate fp32 matmuls (fp32 data only)."""
    if ap.dtype == FP:
        return ap.bitcast(FPR)
    return ap


def emit(ctx: ExitStack, tc: tile.TileContext, io):
    nc = tc.nc
    xT, wqkT, M, bpr, out = io

    stack = []  # (name, free) in creation order; freed strictly LIFO

    def single(shape, dtype, name):
        t, free = tc.tile(shape, dtype, name=name)
        stack.append((name, free))
        return t

    def free_through(name):
        while stack:
            nm, fr = stack.pop()
            fr()
            if nm == name:
                return
        raise KeyError(name)

    # ---------------- PSUM pool: 2 tags x 2 bufs x [128,1024] = 8 banks ----
    psum = ctx.enter_context(tc.tile_pool(name="psum", bufs=2, space="PSUM"))
    _chain = [0]

    def ps_tile(tag=None):
        if tag is None:
            tag = f"ps{_chain[0] & 1}"
            _chain[0] += 1
        return psum.tile([P, N], FP, name=tag, tag=tag)

    # SBUF pools (entered before any single so LIFO holds at ctx exit)
    p_E = ctx.enter_context(tc.tile_pool(name="p_E", bufs=26))
    p_kqs = ctx.enter_context(tc.tile_pool(name="p_kqs", bufs=10))
    p_den = ctx.enter_context(tc.tile_pool(name="p_den", bufs=7))
    p_out = ctx.enter_context(tc.tile_pool(name="p_out", bufs=3))

    # ------------- singles, bottom of stack = longest-lived -------------
    M_all = single([P, H * C], BF, name="M_all")
    M_sb = [M_all[:, ts(h, C)] for h in range(H)]
    bp_sb = single([P, C], FP, name="bp_sb")
    AT_sb = [single([P, N], BF, name=f"AT{h}") for h in range(H)]
    # natkq[j]: [128, 2N] cols 0:N = k natural (t-major 128-col blocks),
    # N:2N = q natural; features c of heads 2j, 2j+1.
    natkq = [single([P, 2 * N], BF, name=f"natkq{j}") for j in range(CT)]
    # qT/kT tile j: [128, N] rows = c_out 128j..128j+127 (heads 2j, 2j+1)
    qT_sb = [single([P, N], BF, name=f"qT{j}") for j in range(CT)]
    kT_sb = [single([P, N], BF, name=f"kT{j}") for j in range(CT)]
    wqkT_all = single([P, CT * 2 * C], BF, name="wqkT_all")
    wqkT_sb = [wqkT_all[:, ts(i, 2 * C)] for i in range(CT)]
    xT_all = single([P, CT * N], BF, name="xT_all")
    xT_sb = [xT_all[:, ts(i, N)] for i in range(CT)]

    # DRAM scratch for the qT/kT -> natural-layout xbar transposes
    qkTd = []
    for j in range(CT):
        t_, _free = tc.tile([2, P, N], BF, space="DRAM", name=f"qkTd{j}")
        qkTd.append(t_)

    # HAM keep-warm scratch: the PE clock-gates to 1.2 GHz after ~3.4us of
    # low activity and needs ~3.4us of sustained work to recover; dummy
    # matmuls on a zeroed tile keep it at 2.4 GHz through the input-DMA
    # window and through exp-paced stretches with no real PE work.
    warm_sb = single([P, 512], BF, name="warm_sb")
    nc.gpsimd.memset(warm_sb[:], 0)

    def dummy_mms(n, tag=None):
        ps = ps_tile(tag)
        for i in range(n):
            nc.tensor.matmul(ps[:, 0:512], lhsT=warm_sb[:, 0:P],
                             rhs=warm_sb[:], start=(i == 0), stop=(i == n - 1))

    # ---------------- batched input DMAs (phase-A inputs first) ---------
    for k in range(CT):
        nc.sync.dma_start(xT_sb[k][:], xT[ts(k, P), :])
        nc.sync.dma_start(wqkT_sb[k][:], wqkT[ts(k, P), :])
    # phase-C inputs follow on the same queue (needed only much later);
    # a second hwdge queue tangles the DMA semaphore ring and stalls the
    # input stream, so everything stays on sync.
    nc.sync.dma_start(M_all[:], M[:])
    nc.sync.dma_start(bp_sb[:], bpr[:])

    # ---------------- projection chains ----------------
    def chain(dst_ap, lhsT_of, rhs_of, width, tag=None):
        """dst_ap = sum_k lhsT_of(k)^T @ rhs_of(k); psum chain + DVE copy."""
        ps = ps_tile(tag)
        for k in range(CT):
            nc.tensor.matmul(
                ps[:, 0:width],
                lhsT=_r(lhsT_of(k)),
                rhs=_r(rhs_of(k)),
                start=(k == 0),
                stop=(k == CT - 1),
            )
        nc.vector.tensor_copy(dst_ap, ps[:, 0:width])

    def qkT_chains(j, tag=None):
        # k chains + q-ch0 first: pair j's scores t=0..3 become ready one
        # chain earlier (they read kT fully but only qT cols 0:512).
        # Returns one thunk per chain so callers can spread them across
        # t-steps: bunching all 4 into one slot monopolizes the 2-buf ps0
        # ring and stalls the next t-step's score matmuls (and the ACT
        # engine behind them) for the whole bunch.
        def one(which, ch):
            cols = slice(512 * ch, 512 * ch + 512)
            dst = (qT_sb if which == 0 else kT_sb)[j][:, cols]
            woff = C * which
            chain(dst,
                  lambda k: wqkT_sb[k][:, woff + 128 * j: woff + 128 * j + P],
                  lambda k: xT_sb[k][:, cols], 512, tag)
        return [lambda w=w, c=c: one(w, c) for w, c in
                [(1, 0), (0, 0), (1, 1), (0, 1)]]

    def make_qkT(j, tag=None):
        for th in qkT_chains(j, tag):
            th()

    def emit_nat_dma(j):
        """qT/kT[j] -> DRAM -> xbar-transposed natural layout natkq[j]."""
        nc.sync.dma_start(qkTd[j][1], kT_sb[j][:])
        nc.sync.dma_start(qkTd[j][0], qT_sb[j][:])
        nc.sync.dma_start_transpose(
            natkq[j][:, 0:N].rearrange("p (t c) -> p t c", c=P),
            qkTd[j][1].rearrange("c (t q) -> c t q", q=P))
        nc.sync.dma_start_transpose(
            natkq[j][:, N:2 * N].rearrange("p (t c) -> p t c", c=P),
            qkTd[j][0].rearrange("c (t q) -> c t q", q=P))

    # warm the PE during the input-DMA window (no data dependencies), then
    # qT/kT for pair 0 up front so scores/exp start as early as possible
    for _ in range(3):
        dummy_mms(8)
    make_qkT(0)
    emit_nat_dma(0)

    # ---------------- phase B + interleaved remaining projections -------
    at_queue = []
    LAG = 18

    def drain_at(n):
        while len(at_queue) > n:
            at_queue.pop(0)()

    at_ps = {}

    # extra PE work emitted inside each pair (fills exp-paced stretches).
    # NB: trace order defines dependencies -- every producer must be
    # emitted before its first reader. natkq[j] is read by the lagged
    # scale ops of pair j; qT/kT[j] by pair j's scores.
    extras = {j: qkT_chains(j + 1, "ps0") + [lambda j=j: emit_nat_dma(j + 1)]
              for j in range(5)}
    # pair 5 has no projection work left; feed the PE dummy matmuls per
    # t-step so HAM stays at 2.4 GHz through the exp-paced final pair
    extras[5] = [lambda: dummy_mms(5, "ps0") for _ in range(7)]

    for j in range(H // 2):
        heads = (2 * j, 2 * j + 1)
        qt, kt = qT_sb[j], kT_sb[j]
        nat3 = natkq[j].rearrange("p (g t c) -> p g t c", g=2, c=P)
        den_t = {h: p_den.tile([P, NT], FP, name="dent") for h in heads}
        rv_t = {h: p_den.tile([P, NT], FP, name="rvt") for h in heads}
        for h in heads:
            at_ps[h] = ps_tile("ps1")
        ext = extras.get(j, [])
        for t in range(NT):
            S = {h: ps_tile("ps0") for h in heads}
            for ch in range(2):
                cols = slice(512 * ch, 512 * ch + 512)
                for h in heads:
                    base = Z * (h & 1)
                    nc.tensor.matmul(
                        S[h][:, cols],
                        lhsT=qt[base:base + Z, ts(t, P)],
                        rhs=kt[base:base + Z, cols],
                        start=True, stop=True,
                    )
            for h in heads:
                E = p_E.tile([P, N], BF, name="Et")
                nc.scalar.activation(
                    E[:], S[h][:], mybir.ActivationFunctionType.Exp,
                    scale=SCALE, accum_out=den_t[h][:, t:t + 1],
                )

                def at_mm(h=h, t=t, E=E, den_t=den_t, rv_t=rv_t, nat3=nat3):
                    nc.vector.reciprocal(rv_t[h][:, t:t + 1],
                                         den_t[h][:, t:t + 1])
                    kqs = p_kqs.tile([P, 2 * Z], BF, name="kqst")
                    nc.vector.tensor_scalar_mul(
                        kqs[:].rearrange("p (g z) -> p g z", g=2),
                        nat3[:, :, t, ts(h & 1, Z)],
                        rv_t[h][:, t:t + 1],
                    )
                    for ch in range(2):
                        cols = slice(512 * ch, 512 * ch + 512)
                        nc.tensor.matmul(
                            at_ps[h][:, cols],
                            lhsT=kqs[:],
                            rhs=E[:, cols],
                            start=(t == 0), stop=(t == NT - 1),
                        )

                at_queue.append(at_mm)
                drain_at(LAG)
            if ext:
                ext.pop(0)()
        for h in heads:
            def at_copy(h=h):
                nc.vector.tensor_copy(AT_sb[h][:], at_ps.pop(h)[:])
            at_queue.append(at_copy)
    drain_at(0)

    free_through("natkq0")  # frees xT, wqkT, kT*, qT*, natkq*

    # ---------------- phase C: fused combine + projection + bias ------
    for t in range(NT):
        F_ps = ps_tile()
        for h in range(H):
            for off, w in CCH:
                nc.tensor.matmul(
                    F_ps[:, off:off + w],
                    lhsT=AT_sb[h][:, ts(t, P)],
                    rhs=M_sb[h][:, off:off + w],
                    start=(h == 0), stop=(h == H - 1),
                )
        o = p_out.tile([P, C], FP, name="outt")
        nc.vector.tensor_add(o[:], F_ps[:, 0:C], bp_sb[:])
        nc.sync.dma_start(out[ts(t, P), :], o[:])

    while stack:
        stack.pop()[1]()


def build():
    nc = bacc.Bacc("TRN2", target_bir_lowering=False, debug=False, num_devices=B)
    xT = nc.dram_tensor("xT", [C, N], BF, kind="ExternalInput").ap()
    wqkT = nc.dram_tensor("wqkT", [C, 2 * C], BF, kind="ExternalInput").ap()
    M = nc.dram_tensor("M", [P, H * C], BF, kind="ExternalInput").ap()
    bpr = nc.dram_tensor("bpr", [P, C], FP, kind="ExternalInput").ap()
    out = nc.dram_tensor("out", [N, C], FP, kind="ExternalOutput").ap()
    with tile.TileContext(nc) as tc, ExitStack() as ctx:
        emit(ctx, tc, (xT, wqkT, M, bpr, out))
    nc.compile()
    return nc


def kernel(x, Wq, Wk, Wp, bp, trace=False, **trace_kwargs):
    global last_results
    x = np.asarray(x, dtype=np.float32)
    Wq = np.asarray(Wq, dtype=np.float32)
    Wk = np.asarray(Wk, dtype=np.float32)
    Wp = np.asarray(Wp, dtype=np.float32)
    bp = np.asarray(bp, dtype=np.float32)

    nc = build()
    bf = ml_dtypes.bfloat16
    wqkTc = np.ascontiguousarray(
        np.concatenate([Wq.T, Wk.T], axis=1)).astype(bf)  # [C, 2C]
    # fused combine+projection weights: M_hT = [Wq_h; Wk_h] @ Wp^T  [2Z, C]
    Wq_h = Wq.reshape(H, Z, C)
    Wk_h = Wk.reshape(H, Z, C)
    W2 = np.concatenate([Wq_h, Wk_h], axis=1)             # [H, 2Z, C]
    M_np = np.einsum("hzc,dc->hzd", W2, Wp)               # [H, 2Z, C]
    Mc = np.ascontiguousarray(
        M_np.transpose(1, 0, 2).reshape(P, H * C)).astype(bf)
    bprc = np.ascontiguousarray(
        np.broadcast_to(bp.reshape(1, C), (P, C)).astype(np.float32))
    in_maps = []
    for b in range(B):
        in_maps.append({
            "xT": np.ascontiguousarray(x[b].T).astype(bf),
            "wqkT": wqkTc, "M": Mc, "bpr": bprc,
        })
    res = bass_utils.run_bass_kernel_spmd(
        nc, in_maps, core_ids=list(range(B)), trace=trace, **trace_kwargs)
    last_results = res
    return np.stack([res.results[b]["out"] for b in range(B)], axis=0)
